# revision 32
# baseline (speedup 1.0000x reference)
"""KAN block (2x KAN layer, dense_mlp) TRN2 Bass kernel — data-parallel on 8 cores.

Full inputs in, full output out. Tokens (B*S = 4096) are sharded 8 ways
(512 per core); weights are replicated.

Device math per KAN layer (out = silu(x) @ wb.T + einsum('nig,oig->no', B(x), ws)):
the 8 cubic B-spline bases on the uniform 12-knot grid are expressed exactly
through 12 one-sided truncated cubes of y=|x| split by sign(x):

    a_j = relu(t_j - y)  (j = 6..11),  u_j = a_j^3 [x>=0],  v_j = a_j^3 [x<0]

folded into the dense weights (truncated-power representation evaluated from
the near side so cube magnitudes stay <= 2.2^3). Each layer is ONE dense
matmul contracting over 13*I (silu + 12 cube features per input dim).

v2 changes vs the 681/773-us baseline:
  - features and weights in fp16 (guaranteed 1 cycle/row PE streaming,
    half the LDWEIGHTS/DMA bytes, DVE 2x/4x packed modes).
  - precision hybrid: the three large-knot cube chains (t=1.4,1.8,2.2),
    which dominate the cancellation-amplified rounding error, run in fp32
    on the Scalar engine (Relu+Square); the three small ones run as packed
    fp16 chains on the Vector engine. Sim rel-err ~7e-3 (gate 2e-2).
  - L1 features generated ONCE (baseline regenerated them per m-group).
  - sign masks as u = pos*c, v = c - u; pos derived from sign(silu).
  - L1 m-groups of 2 PSUM banks, double-buffered (2 tags x 2 bufs), with
    the PE stream software-pipelined one L1 group ahead of L2 consumption.
"""

import numpy as np
from contextlib import ExitStack
from math import comb

import concourse.bass as bass
import concourse.bacc as bacc
import concourse.mybir as mybir
import concourse.tile as tile
from concourse.bass_utils import run_bass_kernel_spmd

F32 = mybir.dt.float32
F16 = mybir.dt.float16
AF = mybir.ActivationFunctionType
ALU = mybir.AluOpType

# Problem constants (hardcoded per contract)
B, S, D, F = 2, 2048, 512, 2048
N_CORES = 8
T = (B * S) // N_CORES          # 512 tokens per core
G_INT, K_ORD = 5, 3
NFEAT = 13                      # [silu, u0..u5, v0..v5]
ACT_JS = (3, 4, 5)              # fp32 chains on ScalarE (t = 1.4, 1.8, 2.2)
GP_VS = (0, 1, 2)               # v-features emitted on GpSimd
G1 = 2                          # layer-1 output tiles per PSUM group
D_T, F_T = D // 128, F // 128   # 4, 16
NG1 = F_T // G1                 # 8 layer-1 m-groups
KT1 = D_T * NFEAT               # 52 layer-1 k-tiles
KT2 = F_T * NFEAT               # 208 layer-2 k-tiles


def knots_f32():
    return (np.arange(-K_ORD, G_INT + K_ORD + 1, dtype=np.float32)
            * np.float32(2.0 / G_INT) - np.float32(1.0))


def fold_maps():
    inv6h3 = 1.0 / (6.0 * (2.0 / G_INT) ** 3)
    M = np.zeros((8, 12))
    for g in range(8):
        for k in range(5):
            M[g, g + k] = ((-1) ** k) * comb(4, k) * inv6h3
    return M[:, 6:12].copy(), M[:, 5::-1].copy()


def fold_weights(wb, ws):
    """wb: (O, I), ws: (O, I, 8) -> (O, I, 13) fp32 augmented weights.

    Device feature signs: ScalarE chains produce c = +a^3, VectorE chains
    produce c = -a^3; u = pos*c and v = c - u inherit c's sign.
    """
    Mu, Mv = fold_maps()
    Wu = np.einsum('oig,gj->oij', ws.astype(np.float64), Mu)
    Wv = np.einsum('oig,gj->oij', ws.astype(np.float64), Mv)
    for j in range(6):
        su = 1.0 if j in ACT_JS else -1.0
        Wu[:, :, j] *= su
        Wv[:, :, j] *= su
    Waug = np.concatenate([wb.astype(np.float64)[:, :, None], Wu, Wv], axis=2)
    return Waug.astype(np.float32)


# L1 m-groups as (start m-tile, width)
GROUPS = ((0, 2), (2, 2), (4, 2), (6, 2),
          (8, 2), (10, 2), (12, 2), (14, 2))


def pack_w1(Waug1):
    """(F, D, 13) -> list of (D_T, 128, NFEAT*width*128) fp16 per group:
    [dtile, k_part, feat*m_free]."""
    out = []
    for s, w in GROUPS:
        A = Waug1[s * 128:(s + w) * 128]        # [w*128m, D, 13]
        A = A.reshape(w * 128, D_T, 128, NFEAT).transpose(1, 2, 3, 0)
        out.append(np.ascontiguousarray(
            A.reshape(D_T, 128, NFEAT * w * 128).astype(np.float16)))
    return out


def pack_w2(Waug2):
    """(D, F, 13) -> (F_T, 128, NFEAT*D) fp16: [ftile, k_part, feat*m_free]."""
    A = Waug2.transpose(1, 2, 0).reshape(F_T, 128, NFEAT * D)
    return np.ascontiguousarray(A.astype(np.float16))


def build_kernel():
    t = knots_f32()
    MW1 = G1 * 128                  # 256 m-cols per L1 weight tile
    nc = bacc.Bacc()

    # knot constants as [128,1] const APs (activation bias operands)
    for j in ACT_JS:
        val = float(t[6 + j])
        ctens = nc.alloc_sbuf_tensor(f"const-knot-{j}", [128, 1], F32)
        nc.gpsimd.memset(ctens.ap(), val)
        nc.const_aps.aps[(F32, val)] = ctens.ap()
    nc.all_engine_barrier()

    xT = nc.declare_dram_parameter("xT", [D, T], F32, isOutput=False)
    w1ts = [nc.declare_dram_parameter(f"w1t{i}",
                                      [D_T, 128, NFEAT * w * 128], F16,
                                      isOutput=False)
            for i, (s, w) in enumerate(GROUPS)]
    w2t = nc.declare_dram_parameter("w2t", [F_T, 128, NFEAT * D], F16,
                                    isOutput=False)
    outT = nc.declare_dram_parameter("outT", [D, T], F32, isOutput=True)

    with ExitStack() as ctx:
        tc = ctx.enter_context(tile.TileContext(nc))
        xpool = ctx.enter_context(tc.tile_pool(name="xp", bufs=1))
        f1pool = ctx.enter_context(tc.tile_pool(name="f1p", bufs=1))
        f2pool = ctx.enter_context(tc.tile_pool(name="f2p", bufs=2))
        scr = ctx.enter_context(tc.tile_pool(name="scr", bufs=2))
        w1pool = ctx.enter_context(tc.tile_pool(name="w1p", bufs=6))
        w2pool = ctx.enter_context(tc.tile_pool(name="w2p", bufs=3))
        opool = ctx.enter_context(tc.tile_pool(name="op", bufs=2))
        pp = ctx.enter_context(tc.tile_pool(name="pp", bufs=1, space="PSUM"))

        # interleave x-tile and first L1 weight-group DMAs so the PE can
        # start the first matmuls as early as possible; the very first
        # weight tile is split in two so its first half lands sooner
        xtiles = []
        w1_first = []
        for dt in range(D_T):
            xt = xpool.tile([128, T], F32, name=f"x{dt}", tag=f"x{dt}")
            nc.sync.dma_start(out=xt, in_=xT[dt * 128:(dt + 1) * 128, :])
            xtiles.append(xt)
            if dt == 0:
                wa0 = w1pool.tile([128, 7 * MW1], F16, name="w1_0_0a",
                                  tag="w1h", bufs=1)
                nc.sync.dma_start(out=wa0, in_=w1ts[0][0, :, 0:7 * MW1])
                wb0 = w1pool.tile([128, 6 * MW1], F16, name="w1_0_0b",
                                  tag="w1g", bufs=1)
                nc.sync.dma_start(out=wb0, in_=w1ts[0][0, :, 7 * MW1:])
                w1_first.append((wa0, wb0))
            else:
                wt = w1pool.tile([128, NFEAT * MW1], F16,
                                 name=f"w1_0_{dt}", tag="w1")
                nc.sync.dma_start(out=wt, in_=w1ts[0][dt, :, :])
                w1_first.append(wt)

        psum2 = [pp.tile([128, T], F32, name=f"ps2_{m}", tag=f"l2p{m}")
                 for m in range(D_T)]

        def gen_features(src, pool, blk, tag_pfx, bufs):
            """13 fp16 feature tiles [sil, u0..u5, v0..v5] from fp32 src."""
            sil = pool.tile([128, T], F16, name=f"sil{blk}",
                            tag=f"{tag_pfx}s", bufs=bufs)
            nc.scalar.activation(sil, src, AF.Silu)
            y = scr.tile([128, T], F32, name=f"y{blk}", tag="y", bufs=2)
            nc.scalar.activation(y, src, AF.Abs)
            pos = scr.tile([128, T], F16, name=f"pos{blk}", tag="pos", bufs=2)
            nc.vector.tensor_scalar(out=pos, in0=sil, scalar1=0.0,
                                    scalar2=None, op0=ALU.is_ge)
            us, vs = [], []
            for j in range(6):
                tj = float(t[6 + j])
                if j in ACT_JS:     # fp32 chain on ScalarE: c = +a^3
                    a = scr.tile([128, T], F32, name=f"a{blk}_{j}", tag="aA",
                                 bufs=2)
                    nc.scalar.activation(a, y, AF.Relu, bias=tj, scale=-1.0)
                    q = scr.tile([128, T], F32, name=f"q{blk}_{j}", tag="qA",
                                 bufs=2)
                    nc.scalar.activation(q, a, AF.Square)
                    c = scr.tile([128, T], F16, name=f"c{blk}_{j}", tag="c",
                                 bufs=4)
                    nc.vector.tensor_tensor(c, q, a, ALU.mult)
                else:               # packed fp16 chain on VectorE: c = -a^3
                    a = scr.tile([128, T], F16, name=f"a{blk}_{j}", tag="aV",
                                 bufs=2)
                    nc.vector.tensor_scalar(out=a, in0=y, scalar1=tj,
                                            scalar2=0.0, op0=ALU.subtract,
                                            op1=ALU.min)
                    aa = scr.tile([128, T], F16, name=f"aa{blk}_{j}",
                                  tag="aaV", bufs=2)
                    nc.vector.tensor_tensor(aa, a, a, ALU.mult)
                    c = scr.tile([128, T], F16, name=f"c{blk}_{j}", tag="c",
                                 bufs=4)
                    nc.vector.tensor_tensor(c, aa, a, ALU.mult)
                u = pool.tile([128, T], F16, name=f"u{blk}_{j}",
                              tag=f"{tag_pfx}u{j}", bufs=bufs)
                nc.vector.tensor_tensor(u, pos, c, ALU.mult)
                v = pool.tile([128, T], F16, name=f"v{blk}_{j}",
                              tag=f"{tag_pfx}v{j}", bufs=bufs)
                eng = nc.gpsimd if j in GP_VS else nc.vector
                eng.tensor_tensor(v, c, u, ALU.subtract)
                us.append(u)
                vs.append(v)
            return [sil] + us + vs

        l1feats = [gen_features(xtiles[dt], f1pool, blk=f"a{dt}",
                                tag_pfx=f"f1_{dt}", bufs=1)
                   for dt in range(D_T)]

        psum1 = {}

        def emit_l1(gi):
            s, width = GROUPS[gi]
            mw = width * 128
            ps = [pp.tile([128, T], F32, name=f"ps1_{gi}_{mi}",
                          tag=f"l1p{mi}", bufs=2) for mi in range(width)]
            psum1[gi] = ps
            if gi == 0:
                wts = w1_first
            else:
                wts = []
                for dt in range(D_T):
                    wt = w1pool.tile([128, NFEAT * mw], F16,
                                     name=f"w1_{gi}_{dt}", tag="w1")
                    nc.sync.dma_start(out=wt, in_=w1ts[gi][dt, :, :])
                    wts.append(wt)
            # mi-major so psum1[0] closes partway through the group and its
            # L2 feature generation overlaps the remaining matmuls
            for mi in range(width):
                for dt in range(D_T):
                    for f in range(NFEAT):
                        kt = dt * NFEAT + f
                        w = wts[dt]
                        if isinstance(w, tuple):
                            w = w[0] if f < 7 else w[1]
                            lo = (f if f < 7 else f - 7) * mw + mi * 128
                        else:
                            lo = f * mw + mi * 128
                        nc.tensor.matmul(
                            ps[mi],
                            lhsT=w[:, lo:lo + 128],
                            rhs=l1feats[dt][f],
                            start=(kt == 0), stop=(kt == KT1 - 1),
                        )

        def emit_l2(gi):
            s, width = GROUPS[gi]
            for mi in range(width):
                g2 = s + mi
                l2f = gen_features(psum1[gi][mi], f2pool, blk=f"b{g2}",
                                   tag_pfx="f2", bufs=2)
                wa = w2pool.tile([128, 7 * D], F16, name=f"w2a_{g2}",
                                 tag="w2a")
                nc.sync.dma_start(out=wa, in_=w2t[g2, :, 0:7 * D])
                wb_ = w2pool.tile([128, 6 * D], F16, name=f"w2b_{g2}",
                                  tag="w2b")
                nc.sync.dma_start(out=wb_, in_=w2t[g2, :, 7 * D:NFEAT * D])
                # for the final k-group, m2-major so psum2 banks close
                # staggered and the output copies overlap the last matmuls
                if g2 == F_T - 1:
                    fm_order = [(f, m2) for m2 in range(D_T)
                                for f in range(NFEAT)]
                else:
                    fm_order = [(f, m2) for f in range(NFEAT)
                                for m2 in range(D_T)]
                for f, m2 in fm_order:
                    kt2 = g2 * NFEAT + f
                    wsl = wa if f < 7 else wb_
                    lo = (f if f < 7 else f - 7) * D + m2 * 128
                    nc.tensor.matmul(
                        psum2[m2],
                        lhsT=wsl[:, lo:lo + 128],
                        rhs=l2f[f],
                        start=(kt2 == 0), stop=(kt2 == KT2 - 1),
                    )
            del psum1[gi]

        emit_l1(0)
        emit_l1(1)
        for gi in range(len(GROUPS)):
            emit_l2(gi)
            if gi + 2 < len(GROUPS):
                emit_l1(gi + 2)

        for m2 in range(D_T):
            ot = opool.tile([128, T], F32, name=f"o{m2}", tag="out")
            if m2 % 2 == 0:
                nc.scalar.activation(ot, psum2[m2], AF.Copy)
            else:
                nc.vector.tensor_copy(ot, psum2[m2])
            nc.sync.dma_start(out=outT[m2 * 128:(m2 + 1) * 128, :], in_=ot)

    nc.finalize()
    return nc


_NC_CACHE = None


def _get_nc():
    global _NC_CACHE
    if _NC_CACHE is None:
        _NC_CACHE = build_kernel()
    return _NC_CACHE


def run(x, w1_base, w1_spline, w2_base, w2_spline, trace=False, **spmd_kwargs):
    x = np.asarray(x, dtype=np.float32)
    xf = np.ascontiguousarray(x.reshape(B * S, D))
    w1p = pack_w1(fold_weights(np.asarray(w1_base), np.asarray(w1_spline)))
    w2p = pack_w2(fold_weights(np.asarray(w2_base), np.asarray(w2_spline)))
    in_maps = []
    for c in range(N_CORES):
        shard = xf[c * T:(c + 1) * T]
        m = {
            "xT": np.ascontiguousarray(shard.T),
            "w2t": w2p,
        }
        for i, arr in enumerate(w1p):
            m[f"w1t{i}"] = arr
        in_maps.append(m)
    nc = _get_nc()
    res = run_bass_kernel_spmd(nc, in_maps, list(range(N_CORES)),
                               trace=trace, **spmd_kwargs)
    outs = [np.asarray(r["outT"]).T for r in res.results]   # each (T, D)
    out = np.concatenate(outs, axis=0).reshape(B, S, D).astype(np.float32)
    return out, res


def kernel(x, grid, w1_base, w1_spline, w2_base, w2_spline):
    out, _ = run(x, w1_base, w1_spline, w2_base, w2_spline)
    return out


# revision 40
# speedup vs baseline: 1.2869x; 1.2869x over previous
"""KAN block (2x KAN layer, dense_mlp) TRN2 Bass kernel — data-parallel on 8 cores.

Full inputs in, full output out. Tokens (B*S = 4096) are sharded 8 ways
(512 per core); weights are replicated.

Device math per KAN layer (out = silu(x) @ wb.T + einsum('nig,oig->no', B(x), ws)):
the 8 cubic B-spline bases on the uniform 12-knot grid are expressed exactly
through 12 one-sided truncated cubes of y=|x| split by sign(x):

    a_j = relu(t_j - y)  (j = 6..11),  u_j = a_j^3 [x>=0],  v_j = a_j^3 [x<0]

folded into the dense weights (truncated-power representation evaluated from
the near side so cube magnitudes stay <= 2.2^3). Each layer is ONE dense
matmul contracting over 13*I (silu + 12 cube features per input dim).

v2 changes vs the 681/773-us baseline:
  - features and weights in fp16 (guaranteed 1 cycle/row PE streaming,
    half the LDWEIGHTS/DMA bytes, DVE 2x/4x packed modes).
  - precision hybrid: the three large-knot cube chains (t=1.4,1.8,2.2),
    which dominate the cancellation-amplified rounding error, run in fp32
    on the Scalar engine (Relu+Square); the three small ones run as packed
    fp16 chains on the Vector engine. Sim rel-err ~7e-3 (gate 2e-2).
  - L1 features generated ONCE (baseline regenerated them per m-group).
  - sign masks as u = pos*c, v = c - u; pos derived from sign(silu).
  - L1 m-groups of 2 PSUM banks, double-buffered (2 tags x 2 bufs), with
    the PE stream software-pipelined one L1 group ahead of L2 consumption.
"""

import numpy as np
from contextlib import ExitStack
from math import comb

import concourse.bass as bass
import concourse.bacc as bacc
import concourse.mybir as mybir
import concourse.tile as tile
from concourse.bass_utils import run_bass_kernel_spmd

F32 = mybir.dt.float32
F16 = mybir.dt.float16
AF = mybir.ActivationFunctionType
ALU = mybir.AluOpType

# Problem constants (hardcoded per contract)
B, S, D, F = 2, 2048, 512, 2048
N_CORES = 8
T = (B * S) // N_CORES          # 512 tokens per core
G_INT, K_ORD = 5, 3
NFEAT = 12                      # [silu, c0, u1..u5, v1..v5]
ACT_JS = (3, 4, 5)              # fp32 chains on ScalarE (t = 1.4, 1.8, 2.2)
GP_VS = (1, 2, 3)               # v-features emitted on GpSimd
G1 = 2                          # layer-1 output tiles per PSUM group
D_T, F_T = D // 128, F // 128   # 4, 16
NG1 = F_T // G1                 # 8 layer-1 m-groups
KT1 = D_T * NFEAT               # 52 layer-1 k-tiles
KT2 = F_T * NFEAT               # 208 layer-2 k-tiles


def knots_f32():
    return (np.arange(-K_ORD, G_INT + K_ORD + 1, dtype=np.float32)
            * np.float32(2.0 / G_INT) - np.float32(1.0))


def fold_maps():
    inv6h3 = 1.0 / (6.0 * (2.0 / G_INT) ** 3)
    M = np.zeros((8, 12))
    for g in range(8):
        for k in range(5):
            M[g, g + k] = ((-1) ** k) * comb(4, k) * inv6h3
    return M[:, 6:12].copy(), M[:, 5::-1].copy()


def fold_weights(wb, ws):
    """wb: (O, I), ws: (O, I, 8) -> (O, I, 13) fp32 augmented weights.

    Device feature signs: ScalarE chains produce c = +a^3, VectorE chains
    produce c = -a^3; u = pos*c and v = c - u inherit c's sign.

    The j=0 (t=0.2) u/v pair is merged into the single unmasked cube c0
    with the sign-symmetric optimal weight (Wu0+Wv0)/2 — its response is
    tiny (cube <= 0.008), and dropping the sign split saves 1/13 of the
    contraction.
    """
    Mu, Mv = fold_maps()
    Wu = np.einsum('oig,gj->oij', ws.astype(np.float64), Mu)
    Wv = np.einsum('oig,gj->oij', ws.astype(np.float64), Mv)
    for j in range(6):
        su = 1.0 if j in ACT_JS else -1.0
        Wu[:, :, j] *= su
        Wv[:, :, j] *= su
    Wm0 = (Wu[:, :, 0:1] + Wv[:, :, 0:1]) / 2.0
    Waug = np.concatenate([wb.astype(np.float64)[:, :, None], Wm0,
                           Wu[:, :, 1:], Wv[:, :, 1:]], axis=2)
    return Waug.astype(np.float32)


# L1 m-groups as (start m-tile, width)
GROUPS = ((0, 2), (2, 2), (4, 2), (6, 2),
          (8, 2), (10, 2), (12, 2), (14, 2))


def pack_w1(Waug1):
    """(F, D, 13) -> list of (D_T, 128, NFEAT*width*128) fp16 per group:
    [dtile, k_part, feat*m_free]."""
    out = []
    for s, w in GROUPS:
        A = Waug1[s * 128:(s + w) * 128]        # [w*128m, D, 13]
        A = A.reshape(w * 128, D_T, 128, NFEAT).transpose(1, 2, 3, 0)
        out.append(np.ascontiguousarray(
            A.reshape(D_T, 128, NFEAT * w * 128).astype(np.float16)))
    return out


def pack_w2(Waug2):
    """(D, F, 13) -> (F_T, 128, NFEAT*D) fp16: [ftile, k_part, feat*m_free]."""
    A = Waug2.transpose(1, 2, 0).reshape(F_T, 128, NFEAT * D)
    return np.ascontiguousarray(A.astype(np.float16))


def build_kernel():
    t = knots_f32()
    MW1 = G1 * 128                  # 256 m-cols per L1 weight tile
    nc = bacc.Bacc()

    # knot constants as [128,1] const APs (activation bias operands)
    for j in ACT_JS:
        val = float(t[6 + j])
        ctens = nc.alloc_sbuf_tensor(f"const-knot-{j}", [128, 1], F32)
        nc.gpsimd.memset(ctens.ap(), val)
        nc.const_aps.aps[(F32, val)] = ctens.ap()
    nc.all_engine_barrier()

    xT = nc.declare_dram_parameter("xT", [D, T], F32, isOutput=False)
    w1ts = [nc.declare_dram_parameter(f"w1t{i}",
                                      [D_T, 128, NFEAT * w * 128], F16,
                                      isOutput=False)
            for i, (s, w) in enumerate(GROUPS)]
    w2t = nc.declare_dram_parameter("w2t", [F_T, 128, NFEAT * D], F16,
                                    isOutput=False)
    outT = nc.declare_dram_parameter("outT", [D, T], F32, isOutput=True)

    with ExitStack() as ctx:
        tc = ctx.enter_context(tile.TileContext(nc))
        xpool = ctx.enter_context(tc.tile_pool(name="xp", bufs=1))
        f1pool = ctx.enter_context(tc.tile_pool(name="f1p", bufs=1))
        f2pool = ctx.enter_context(tc.tile_pool(name="f2p", bufs=2))
        scr = ctx.enter_context(tc.tile_pool(name="scr", bufs=2))
        w1pool = ctx.enter_context(tc.tile_pool(name="w1p", bufs=6))
        w2pool = ctx.enter_context(tc.tile_pool(name="w2p", bufs=3))
        opool = ctx.enter_context(tc.tile_pool(name="op", bufs=2))
        pp = ctx.enter_context(tc.tile_pool(name="pp", bufs=1, space="PSUM"))

        # interleave x-tile and first L1 weight-group DMAs so the PE can
        # start the first matmuls as early as possible; the very first
        # weight tile is split in two so its first half lands sooner
        xtiles = []
        w1_first = []
        for dt in range(D_T):
            xt = xpool.tile([128, T], F32, name=f"x{dt}", tag=f"x{dt}")
            nc.sync.dma_start(out=xt, in_=xT[dt * 128:(dt + 1) * 128, :])
            xtiles.append(xt)
            if dt == 0:
                wa0 = w1pool.tile([128, 6 * MW1], F16, name="w1_0_0a",
                                  tag="w1h", bufs=1)
                nc.sync.dma_start(out=wa0, in_=w1ts[0][0, :, 0:6 * MW1])
                wb0 = w1pool.tile([128, 6 * MW1], F16, name="w1_0_0b",
                                  tag="w1g", bufs=1)
                nc.sync.dma_start(out=wb0, in_=w1ts[0][0, :, 6 * MW1:])
                w1_first.append((wa0, wb0))
            else:
                wt = w1pool.tile([128, NFEAT * MW1], F16,
                                 name=f"w1_0_{dt}", tag="w1")
                nc.sync.dma_start(out=wt, in_=w1ts[0][dt, :, :])
                w1_first.append(wt)

        psum2 = [pp.tile([128, T], F32, name=f"ps2_{m}", tag=f"l2p{m}")
                 for m in range(D_T)]

        def gen_features(src, pool, blk, tag_pfx, bufs):
            """13 fp16 feature tiles [sil, u0..u5, v0..v5] from fp32 src."""
            sil = pool.tile([128, T], F16, name=f"sil{blk}",
                            tag=f"{tag_pfx}s", bufs=bufs)
            nc.scalar.activation(sil, src, AF.Silu)
            y = scr.tile([128, T], F32, name=f"y{blk}", tag="y", bufs=2)
            nc.scalar.activation(y, src, AF.Abs)
            pos = scr.tile([128, T], F16, name=f"pos{blk}", tag="pos", bufs=2)
            nc.vector.tensor_scalar(out=pos, in0=sil, scalar1=0.0,
                                    scalar2=None, op0=ALU.is_ge)
            us, vs = [], []
            for j in range(6):
                tj = float(t[6 + j])
                if j in ACT_JS:     # fp32 chain on ScalarE: c = +a^3
                    a = scr.tile([128, T], F32, name=f"a{blk}_{j}", tag="aA",
                                 bufs=2)
                    nc.scalar.activation(a, y, AF.Relu, bias=tj, scale=-1.0)
                    q = scr.tile([128, T], F32, name=f"q{blk}_{j}", tag="qA",
                                 bufs=2)
                    nc.scalar.activation(q, a, AF.Square)
                    c = scr.tile([128, T], F16, name=f"c{blk}_{j}", tag="c",
                                 bufs=4)
                    nc.vector.tensor_tensor(c, q, a, ALU.mult)
                else:               # packed fp16 chain on VectorE: c = -a^3
                    a = scr.tile([128, T], F16, name=f"a{blk}_{j}", tag="aV",
                                 bufs=2)
                    nc.vector.tensor_scalar(out=a, in0=y, scalar1=tj,
                                            scalar2=0.0, op0=ALU.subtract,
                                            op1=ALU.min)
                    aa = scr.tile([128, T], F16, name=f"aa{blk}_{j}",
                                  tag="aaV", bufs=2)
                    nc.vector.tensor_tensor(aa, a, a, ALU.mult)
                    if j == 0:
                        c = pool.tile([128, T], F16, name=f"c{blk}_{j}",
                                      tag=f"{tag_pfx}c0", bufs=bufs)
                    else:
                        c = scr.tile([128, T], F16, name=f"c{blk}_{j}",
                                     tag="c", bufs=4)
                    nc.vector.tensor_tensor(c, aa, a, ALU.mult)
                if j == 0:      # merged feature: the unmasked cube itself
                    us.append(c)
                    continue
                u = pool.tile([128, T], F16, name=f"u{blk}_{j}",
                              tag=f"{tag_pfx}u{j}", bufs=bufs)
                nc.vector.tensor_tensor(u, pos, c, ALU.mult)
                v = pool.tile([128, T], F16, name=f"v{blk}_{j}",
                              tag=f"{tag_pfx}v{j}", bufs=bufs)
                eng = nc.gpsimd if j in GP_VS else nc.vector
                eng.tensor_tensor(v, c, u, ALU.subtract)
                us.append(u)
                vs.append(v)
            return [sil] + us + vs

        l1feats = [gen_features(xtiles[dt], f1pool, blk=f"a{dt}",
                                tag_pfx=f"f1_{dt}", bufs=1)
                   for dt in range(D_T)]

        psum1 = {}

        def emit_l1(gi):
            s, width = GROUPS[gi]
            mw = width * 128
            ps = [pp.tile([128, T], F32, name=f"ps1_{gi}_{mi}",
                          tag=f"l1p{mi}", bufs=2) for mi in range(width)]
            psum1[gi] = ps
            if gi == 0:
                wts = w1_first
            else:
                wts = []
                for dt in range(D_T):
                    wt = w1pool.tile([128, NFEAT * mw], F16,
                                     name=f"w1_{gi}_{dt}", tag="w1")
                    nc.sync.dma_start(out=wt, in_=w1ts[gi][dt, :, :])
                    wts.append(wt)
            # mi-major so psum1[0] closes partway through the group and its
            # L2 feature generation overlaps the remaining matmuls
            for mi in range(width):
                for dt in range(D_T):
                    for f in range(NFEAT):
                        kt = dt * NFEAT + f
                        w = wts[dt]
                        if isinstance(w, tuple):
                            w = w[0] if f < 6 else w[1]
                            lo = (f if f < 6 else f - 6) * mw + mi * 128
                        else:
                            lo = f * mw + mi * 128
                        nc.tensor.matmul(
                            ps[mi],
                            lhsT=w[:, lo:lo + 128],
                            rhs=l1feats[dt][f],
                            start=(kt == 0), stop=(kt == KT1 - 1),
                        )

        def emit_l2(gi):
            s, width = GROUPS[gi]
            for mi in range(width):
                g2 = s + mi
                l2f = gen_features(psum1[gi][mi], f2pool, blk=f"b{g2}",
                                   tag_pfx="f2", bufs=2)
                wa = w2pool.tile([128, 6 * D], F16, name=f"w2a_{g2}",
                                 tag="w2a")
                nc.sync.dma_start(out=wa, in_=w2t[g2, :, 0:6 * D])
                wb_ = w2pool.tile([128, 6 * D], F16, name=f"w2b_{g2}",
                                  tag="w2b")
                nc.sync.dma_start(out=wb_, in_=w2t[g2, :, 6 * D:NFEAT * D])
                # for the final k-group, m2-major so psum2 banks close
                # staggered and the output copies overlap the last matmuls
                if g2 == F_T - 1:
                    fm_order = [(f, m2) for m2 in range(D_T)
                                for f in range(NFEAT)]
                else:
                    fm_order = [(f, m2) for f in range(NFEAT)
                                for m2 in range(D_T)]
                for f, m2 in fm_order:
                    kt2 = g2 * NFEAT + f
                    wsl = wa if f < 6 else wb_
                    lo = (f if f < 6 else f - 6) * D + m2 * 128
                    nc.tensor.matmul(
                        psum2[m2],
                        lhsT=wsl[:, lo:lo + 128],
                        rhs=l2f[f],
                        start=(kt2 == 0), stop=(kt2 == KT2 - 1),
                    )
            del psum1[gi]

        emit_l1(0)
        emit_l1(1)
        for gi in range(len(GROUPS)):
            emit_l2(gi)
            if gi + 2 < len(GROUPS):
                emit_l1(gi + 2)

        for m2 in range(D_T):
            ot = opool.tile([128, T], F32, name=f"o{m2}", tag="out")
            if m2 % 2 == 0:
                nc.scalar.activation(ot, psum2[m2], AF.Copy)
            else:
                nc.vector.tensor_copy(ot, psum2[m2])
            nc.sync.dma_start(out=outT[m2 * 128:(m2 + 1) * 128, :], in_=ot)

    nc.finalize()
    return nc


_NC_CACHE = None


def _get_nc():
    global _NC_CACHE
    if _NC_CACHE is None:
        _NC_CACHE = build_kernel()
    return _NC_CACHE


def run(x, w1_base, w1_spline, w2_base, w2_spline, trace=False, **spmd_kwargs):
    x = np.asarray(x, dtype=np.float32)
    xf = np.ascontiguousarray(x.reshape(B * S, D))
    w1p = pack_w1(fold_weights(np.asarray(w1_base), np.asarray(w1_spline)))
    w2p = pack_w2(fold_weights(np.asarray(w2_base), np.asarray(w2_spline)))
    in_maps = []
    for c in range(N_CORES):
        shard = xf[c * T:(c + 1) * T]
        m = {
            "xT": np.ascontiguousarray(shard.T),
            "w2t": w2p,
        }
        for i, arr in enumerate(w1p):
            m[f"w1t{i}"] = arr
        in_maps.append(m)
    nc = _get_nc()
    res = run_bass_kernel_spmd(nc, in_maps, list(range(N_CORES)),
                               trace=trace, **spmd_kwargs)
    outs = [np.asarray(r["outT"]).T for r in res.results]   # each (T, D)
    out = np.concatenate(outs, axis=0).reshape(B, S, D).astype(np.float32)
    return out, res


def kernel(x, grid, w1_base, w1_spline, w2_base, w2_spline):
    out, _ = run(x, w1_base, w1_spline, w2_base, w2_spline)
    return out
